# revision 1
# baseline (speedup 1.0000x reference)
"""Trainium2 Bass kernel for DegradationRectifyNet block (CSWin-style window
attention + LePE depthwise conv + code-conditioned LN/MLP).

Data-parallel over batch B=8 across 8 NeuronCores; one image per core.
On-chip everything is channel-major (C on partitions, tokens on free dim).

Per-core pipeline:
  LN1 stats via all-ones PE matmul (partition reduction, broadcast-redundant),
  LN affine + z-shift folded into transposed qkv weights; window attention with
  S^T (k on partitions) so softmax denominators come from a ones-column
  appended to the v stationary; exp on ACT with the 1/sqrt(d) scale fused; the
  per-query division is done in a transposed frame via PE transposes; LePE
  depthwise 3x3 runs as 9 block-diagonal PE matmuls over zero-padded window
  images (branch-1 windows stored transposed so both branches share geometry),
  and attention output accumulates directly into the conv PSUM. Proj and the
  gelu MLP are fp32r matmuls with biases/residuals fused into
  scalar_tensor_tensor epilogues.

PE quadrant constraints (operand partition bases must be 32-aligned) are
handled by: (a) computing QK^T with 32-row contractions against k copies that
have the other head of each pair zeroed (kE/kO), and (b) padding AV output
stripes to 32 partitions with explicit zero columns in the stationary.
"""

import numpy as np

import concourse.bass as bass
import concourse.bacc as bacc
import concourse.tile as tile
from concourse import mybir
from concourse import bass_utils
from concourse.masks import make_identity
from contextlib import ExitStack

F32 = mybir.dt.float32
F32R = mybir.dt.float32r
AF = mybir.ActivationFunctionType
OP = mybir.AluOpType

B = 8
C = 128          # channels
H = W = 64
L = H * W        # 4096 tokens
SS = 8           # split size
CB = C // 2      # branch channels = 64
NH = 4           # heads per branch
D = CB // NH     # head dim = 16
HID = 4 * C      # 512
CHZ = 512
NT = 8           # token tiles
TT = 512         # tokens per tile
NWIN = 8         # windows per branch
EPS = 1e-5

FAST = True  # float32r (pre-rounded) tensors for the big matmuls (N>=256)
FR = F32R if FAST else F32


def _r(ap):
    return ap


def _rs(ap):
    return ap


INPUT_NAMES = [
    "x", "z", "ln1_g", "ln1_b", "ln2_g", "ln2_b", "Wz1", "Wz2", "Wqkv",
    "Wproj", "bproj", "lepe_w0", "lepe_b0", "lepe_w1", "lepe_b1",
    "W1", "b1", "W2", "b2",
]
INPUT_SHAPES = {
    "x": [C, L], "z": [CHZ],
    "ln1_g": [C], "ln1_b": [C], "ln2_g": [C], "ln2_b": [C],
    "Wz1": [C, CHZ], "Wz2": [C, CHZ], "Wqkv": [3 * C, C],
    "Wproj": [C, C], "bproj": [C],
    "lepe_w0": [CB, 1, 3, 3], "lepe_b0": [CB],
    "lepe_w1": [CB, 1, 3, 3], "lepe_b1": [CB],
    "W1": [HID, C], "b1": [HID], "W2": [C, HID], "b2": [C],
}


def emit(ctx: ExitStack, tc: tile.TileContext, io: dict):
    nc = tc.nc

    consts = ctx.enter_context(tc.tile_pool(name="consts", bufs=1))
    wpool = ctx.enter_context(tc.tile_pool(name="wpool", bufs=1))
    big = ctx.enter_context(tc.tile_pool(name="big", bufs=1))
    stat = ctx.enter_context(tc.tile_pool(name="stat", bufs=2))
    h1p = ctx.enter_context(tc.tile_pool(name="h1p", bufs=2))
    padp = ctx.enter_context(tc.tile_pool(name="padp", bufs=2))
    expp = ctx.enter_context(tc.tile_pool(name="expp", bufs=2))
    v4p = ctx.enter_context(tc.tile_pool(name="v4p", bufs=2))
    epip = ctx.enter_context(tc.tile_pool(name="epip", bufs=2))
    kstp = ctx.enter_context(tc.tile_pool(name="kstp", bufs=1))
    gelp = ctx.enter_context(tc.tile_pool(name="gelp", bufs=1))
    outp = ctx.enter_context(tc.tile_pool(name="outp", bufs=1))

    # one PSUM pool, 4 tags totalling exactly 8 banks:
    #   big4 (4): QK scores / MLP hidden / LN stats pair / epilogue transpose
    #   bank1 (2, rotating): linear outs, weight transposes, v-prep, AV h3, vecs
    #   conv (1): LePE accumulate
    psum = ctx.enter_context(tc.tile_pool(name="psum", bufs=1, space="PSUM"))

    def ps_big4(name):
        return psum.tile([128, 2048], F32, tag="big4", bufs=1, name=name)

    def ps_bank1(shape, name):
        return psum.tile(shape, F32, tag="bank1", bufs=2, name=name)

    dma = nc.sync.dma_start
    mm = nc.tensor.matmul

    # ---------------- constants ----------------
    ident = consts.tile([128, 128], F32, tag="ident")
    make_identity(nc, ident[:])
    identR = consts.tile([128, 128], FR, tag="identR")
    nc.vector.tensor_copy(identR[:], ident[:])
    # 64x64 identity living at partitions 64:128 (for branch-1 v transposes)
    ident2f = consts.tile([128, 64], F32, tag="ident2f")
    nc.gpsimd.memset(ident2f[:], 0.0)
    nc.gpsimd.affine_select(
        out=ident2f[:], in_=ident2f[:], compare_op=OP.not_equal, fill=1.0,
        base=-64, pattern=[[-1, 64]], channel_multiplier=1,
    )
    ident2 = consts.tile([128, 64], FR, tag="ident2")
    nc.vector.tensor_copy(ident2[:], ident2f[:])
    # fp32 zero source for initializing f32r tiles (memset on f32r is illegal)
    zst = consts.tile([128, 660], F32, tag="zst")
    nc.gpsimd.memset(zst[:], 0.0)
    ones_st = consts.tile([128, 128], F32, tag="ones_st")
    nc.gpsimd.memset(ones_st[:], 1.0)
    ones_rowf = consts.tile([1, TT], F32, tag="ones_rowf")
    nc.gpsimd.memset(ones_rowf[:], 1.0)
    ones_row = consts.tile([1, TT], FR, tag="ones_row")
    nc.vector.tensor_copy(ones_row[:], ones_rowf[:])
    epscol = consts.tile([128, 1], F32, tag="epscol")
    nc.gpsimd.memset(epscol[:], EPS)

    def col(name):
        t = consts.tile([128, 1], F32, tag="col_" + name)
        dma(t[:], io[name].unsqueeze(1))
        return t

    g1c = col("ln1_g")
    bln1 = col("ln1_b")
    g2c = col("ln2_g")
    bln2 = col("ln2_b")
    bprojc = col("bproj")
    b2c = col("b2")

    b1cols = consts.tile([128, 4], F32, tag="b1cols")
    for h in range(4):
        dma(b1cols[:, h : h + 1], io["b1"][128 * h : 128 * h + 128].unsqueeze(1))
    zcols = consts.tile([128, 4], F32, tag="zcols")
    for k in range(4):
        dma(zcols[:, k : k + 1], io["z"][128 * k : 128 * k + 128].unsqueeze(1))

    lepe_stage = consts.tile([1, 128], F32, tag="lepe_stage")
    dma(lepe_stage[0:1, 0:CB], io["lepe_b0"].unsqueeze(0))
    dma(lepe_stage[0:1, CB:128], io["lepe_b1"].unsqueeze(0))
    lepe_row = consts.tile([1, 128], FR, tag="lepe_row")
    nc.vector.tensor_copy(lepe_row[:], lepe_stage[:])

    # conv tap weights: rows 0:64 branch-0 taps (a,b); rows 64:128 branch-1
    # taps transposed (branch-1 window images are stored transposed)
    wcomb = consts.tile([128, 9], F32, tag="wcomb")
    dma(wcomb[0:CB, :], io["lepe_w0"].rearrange("c o a b -> c (o a b)"))
    wstage = consts.tile([64, 9], F32, tag="wstage")
    dma(wstage[:], io["lepe_w1"].rearrange("c o a b -> c (o a b)"))
    nc.gpsimd.tensor_copy(
        wcomb[CB:128, :].rearrange("c (b a) -> c b a", b=3),
        wstage[:].rearrange("c (a b) -> c a b", a=3).transpose([0, 2, 1]),
    )

    # ---------------- weight transposes ----------------
    def load_transposed(dst_ap, src_ap, scale_col=None, copy_dst=None):
        wt = wpool.tile([128, 128], F32, tag="wtmp")
        dma(wt[:], src_ap)
        pt = ps_bank1([128, 128], "ptw")
        nc.tensor.transpose(pt[:], wt[:], ident[:])
        if scale_col is not None:
            nc.vector.tensor_scalar_mul(dst_ap, pt[:], scale_col)
        else:
            nc.vector.tensor_copy(dst_ap, pt[:])
        if copy_dst is not None:
            nc.vector.tensor_copy(copy_dst, pt[:])

    WqkvT = wpool.tile([128, 3 * C], F32, tag="WqkvT")
    WgT = wpool.tile([128, 3 * C], FR, tag="WgT")
    for j in range(3):
        load_transposed(
            WgT[:, 128 * j : 128 * j + 128],
            io["Wqkv"][128 * j : 128 * j + 128, :],
            scale_col=g1c[:],
            copy_dst=WqkvT[:, 128 * j : 128 * j + 128],
        )

    W1T = wpool.tile([128, HID], F32, tag="W1T")
    W1gT = wpool.tile([128, HID], FR, tag="W1gT")
    for h in range(4):
        load_transposed(
            W1gT[:, 128 * h : 128 * h + 128],
            io["W1"][128 * h : 128 * h + 128, :],
            scale_col=g2c[:],
            copy_dst=W1T[:, 128 * h : 128 * h + 128],
        )

    W2T = wpool.tile([128, HID], FR, tag="W2T")
    for h in range(4):
        load_transposed(
            W2T[:, 128 * h : 128 * h + 128], io["W2"][:, 128 * h : 128 * h + 128]
        )

    WpT = wpool.tile([128, C], FR, tag="WpT")
    load_transposed(WpT[:], io["Wproj"][:, :])

    Wz1T = wpool.tile([128, CHZ], F32, tag="Wz1T")
    Wz2T = wpool.tile([128, CHZ], F32, tag="Wz2T")
    for k in range(4):
        load_transposed(
            Wz1T[:, 128 * k : 128 * k + 128], io["Wz1"][:, 128 * k : 128 * k + 128]
        )
        load_transposed(
            Wz2T[:, 128 * k : 128 * k + 128], io["Wz2"][:, 128 * k : 128 * k + 128]
        )

    # ---------------- z-conditioned bias columns ----------------
    def zbias(WzT, blnc, tag):
        zp = ps_bank1([128, 1], "zp")
        for k in range(4):
            mm(
                zp[:], WzT[:, 128 * k : 128 * k + 128], zcols[:, k : k + 1],
                start=(k == 0), stop=(k == 3),
            )
        bz = consts.tile([128, 1], F32, tag=tag)
        nc.vector.tensor_add(bz[:], zp[:], blnc[:])
        return bz

    bz1 = zbias(Wz1T, bln1, "bz1")
    bz2 = zbias(Wz2T, bln2, "bz2")

    w0cols = consts.tile([128, 3], F32, tag="w0cols")
    for j in range(3):
        wp = ps_bank1([128, 1], "wp")
        mm(wp[:], WqkvT[:, 128 * j : 128 * j + 128], bz1[:], start=True, stop=True)
        nc.vector.tensor_copy(w0cols[:, j : j + 1], wp[:])
    gbcols = consts.tile([128, 4], F32, tag="gbcols")
    for h in range(4):
        wp = ps_bank1([128, 1], "wp")
        mm(wp[:], W1T[:, 128 * h : 128 * h + 128], bz2[:], start=True, stop=True)
        nc.vector.tensor_add(gbcols[:, h : h + 1], wp[:], b1cols[:, h : h + 1])

    v4f = consts.tile([128, TT], F32, tag="v4f")
    nc.gpsimd.memset(v4f[:], 0.0)
    nc.vector.memset(
        v4f[:].rearrange("p (s w) -> p s w", s=16)[:, :, 16:17], 1.0
    )
    v4t = consts.tile([128, TT], FR, tag="v4t")
    nc.vector.tensor_copy(v4t[:], v4f[:])

    diags = []
    for k in range(9):
        dg = consts.tile([128, 128], FR, tag=f"diag{k}")
        nc.vector.tensor_scalar_mul(dg[:], ident[:], wcomb[:, k : k + 1])
        diags.append(dg)

    # ---------------- big activation buffers ----------------
    xT = big.tile([128, L], F32, tag="xT")
    for t in range(NT):
        dma(xT[:, TT * t : TT * t + TT], io["x"][:, TT * t : TT * t + TT])
    qT = big.tile([128, L], FR, tag="qT")
    kT = big.tile([128, L], FR, tag="kT")
    vT = big.tile([128, L], FR, tag="vT")
    # 4 persistent staging slabs for QK^T stationaries: slab h keeps only
    # head h's 16 rows live per branch (rows 64*br+16*h), rest stay zero, so
    # the 64-row contraction reads start at partition base 0/64.
    kst4 = [
        kstp.tile([128, TT], FR, tag=f"kst{i}", name=f"kst{i}")
        for i in range(NH)
    ]
    for i in range(NH):
        nc.vector.tensor_copy(kst4[i][:], zst[:, 0:TT])
    cat = big.tile([128, L], FR, tag="cat")
    xf2 = big.tile([128, L], F32, tag="xf2")

    # ---------------- layernorm over channel (partition) dim ----------------
    def layernorm_tile(src, t, tag):
        sl = slice(TT * t, TT * t + TT)
        xt = src[:, sl]
        xsq = stat.tile([128, TT], F32, tag="xsq")
        nc.gpsimd.tensor_mul(xsq[:], xt, xt)
        s12 = ps_big4("s12")
        s1 = s12[:, 0:TT]
        s2 = s12[:, TT : 2 * TT]
        mm(s1, _rs(ones_st[:]), _rs(xt), start=True, stop=True)
        mm(s2, _rs(ones_st[:]), _rs(xsq[:]), start=True, stop=True)
        m = stat.tile([128, TT], F32, tag="m", bufs=1)
        nc.vector.tensor_scalar_mul(m[:], s1, 1.0 / C)
        msq = stat.tile([128, TT], F32, tag="msq", bufs=1)
        nc.gpsimd.tensor_mul(msq[:], m[:], m[:])
        var = stat.tile([128, TT], F32, tag="var", bufs=1)
        nc.vector.scalar_tensor_tensor(
            var[:], s2, 1.0 / C, msq[:], op0=OP.mult, op1=OP.subtract
        )
        sd = stat.tile([128, TT], F32, tag="sd", bufs=1)
        nc.scalar.activation(sd[:], var[:], AF.Sqrt, bias=epscol[:], scale=1.0)
        r = stat.tile([128, TT], F32, tag="r", bufs=1)
        nc.vector.reciprocal(r[:], sd[:])
        xc = stat.tile([128, TT], F32, tag="xc", bufs=1)
        nc.vector.tensor_sub(xc[:], xt, m[:])
        hn = h1p.tile([128, TT], FR, tag="hn")
        nc.vector.tensor_mul(hn[:], xc[:], r[:])
        return hn

    # ---------------- phase 1: LN1 + qkv ----------------
    for t in range(NT):
        sl = slice(TT * t, TT * t + TT)
        hn = layernorm_tile(xT, t, "ln1")
        for j, dstT in enumerate((qT, kT, vT)):
            qp = ps_bank1([128, TT], "qp")
            mm(
                qp[:], _r(WgT[:, 128 * j : 128 * j + 128]), _r(hn[:]),
                start=True, stop=True,
            )
            nc.vector.tensor_scalar_add(dstT[:, sl], qp[:], w0cols[:, j : j + 1])

    # window access patterns -------------------------------------------------
    # branch 0: vertical strip window j = cols [8j,8j+8); raster (h, w)
    # branch 1: horizontal strip window j, stored transposed; raster (w, h)
    def win_ap(src, br, j, p0, psz):
        a3 = src[p0 : p0 + psz, :].rearrange("c (h w) -> c h w", h=H)
        if br == 0:
            return a3[:, :, SS * j : SS * j + SS]
        return a3[:, SS * j : SS * j + SS, :].transpose([0, 2, 1])

    def chunk_ap(src, br, j, c, p0, psz):
        # 128-token k-chunk c of window j; br0 h-major strips, br1 h-major rows
        a3 = src[p0 : p0 + psz, :].rearrange("c (h w) -> c h w", h=H)
        if br == 0:
            return a3[:, 16 * c : 16 * c + 16, SS * j : SS * j + SS]
        return a3[:, SS * j + 2 * c : SS * j + 2 * c + 2, :]

    # ---------------- attention + lepe, one window pair per j ----------------
    for j in range(NWIN):
        # LePE: zero-padded q window images (66 x 10), both branches stacked
        pad = padp.tile([128, 660], FR, tag="pad")
        nc.vector.tensor_copy(pad[:], zst[:])
        pad3 = pad[:].rearrange("c (h w) -> c h w", h=66)
        nc.gpsimd.tensor_copy(pad3[0:CB, 1:65, 1:9], win_ap(qT, 0, j, 0, CB))
        nc.gpsimd.tensor_copy(pad3[CB:128, 1:65, 1:9], win_ap(qT, 1, j, CB, CB))
        vst = v4p.tile([128, TT], FR, tag="vst")
        dma(
            vst[0:CB, :].rearrange("c (a b) -> c a b", a=64),
            win_ap(vT, 0, j, 0, CB),
        )
        dma(vst[CB:128, :], vT[CB:128, TT * j : TT * j + TT])
        convp = psum.tile([128, TT], F32, tag="conv", bufs=1, name="convp")
        taps = [(a, b) for a in (-1, 0, 1) for b in (-1, 0, 1)]
        for idx, (a, b) in enumerate(taps):
            src = pad3[:, 1 + a : 65 + a, 1 + b : 9 + b]
            mm(
                convp[:], _r(diags[3 * (a + 1) + (b + 1)][:]), _r(src),
                start=(idx == 0), stop=False,
            )
        mm(convp[:], _r(lepe_row[:]), _r(ones_row[:]), start=False, stop=True)
        lepe = padp.tile([128, TT], F32, tag="lepe")
        nc.vector.tensor_copy(lepe[:], convp[:])

        for br in range(2):
            p0 = CB * br
            # v': token-major v (via PE transpose), 32-wide head slots:
            # cols [0:16) v, col 16 ones (denominator), cols [17:32) zero
            vps = psum.tile([128, 256], FR, tag="bank1", bufs=2, name="vps")
            idv = identR[0:CB, 0:CB] if br == 0 else ident2[CB:128, :]
            for c in range(4):
                mm(
                    vps[:, 64 * c : 64 * c + 64],
                    vst[p0 : p0 + CB, 128 * c : 128 * c + 128],
                    idv,
                    is_transpose=True,
                    start=(c == 0), stop=(c == 3),
                )
            v4 = v4p.tile([128, TT], FR, tag="v4")
            nc.vector.tensor_copy(v4[:], v4t[:])
            v4v = v4[:].rearrange("p (c h s) -> p c h s", c=4, h=4)
            nc.vector.tensor_copy(
                v4v[:, :, :, 0:16],
                vps[:].rearrange("p (c h d) -> p c h d", c=4, h=4),
            )

            # fp32r matmuls cannot write PSUM at a partition offset, so each
            # head's AV accumulates at offset 0 and is copied to its A stripe.
            A = epip.tile([128, TT], F32, tag="A")
            for h in range(NH):
                hp0 = p0 + D * h
                kst = kst4[h]
                if br == 0:
                    dma(
                        kst[hp0 : hp0 + D, :].rearrange("c (a b) -> c a b", a=64),
                        win_ap(kT, 0, j, hp0, D),
                    )
                else:
                    dma(
                        kst[hp0 : hp0 + D, :],
                        kT[hp0 : hp0 + D, TT * j : TT * j + TT],
                    )
                sp = ps_big4("sp")
                for c in range(4):
                    mm(
                        sp[:, TT * c : TT * c + TT],
                        _r(kst[p0 : p0 + CB, 128 * c : 128 * c + 128]),
                        _r(win_ap(qT, br, j, p0, CB)),
                        start=True, stop=True,
                    )
                es = expp.tile([128, 4 * TT], FR, tag="expS")
                nc.scalar.activation(es[:], sp[:], AF.Exp, scale=float(D) ** -0.5)
                avh = ps_bank1([32, TT], "avh")
                for c in range(4):
                    mm(
                        avh[:],
                        _r(v4[:, 128 * c + 32 * h : 128 * c + 32 * h + 32]),
                        _r(es[:, TT * c : TT * c + TT]),
                        start=(c == 0), stop=(c == 3),
                    )
                nc.vector.tensor_copy(A[32 * h : 32 * h + 32, :], avh[:])

            # epilogue: transpose -> divide by denominators -> transpose back
            Tbig = ps_big4("Tbig")
            Tp = Tbig[:, 0:TT]
            for c in range(4):
                mm(
                    Tp[:, 128 * c : 128 * c + 128],
                    A[:, 128 * c : 128 * c + 128],
                    ident[:],
                    is_transpose=True,
                    start=(c == 0), stop=(c == 3),
                )
            Tv = Tp.rearrange("p (c h s) -> p c h s", c=4, h=4)
            R = epip.tile([128, 16], F32, tag="R")
            Rv = R[:].rearrange("p (c h) -> p c h", c=4)
            nc.vector.reciprocal(Rv[:, :, :], Tv[:, :, :, 16])
            E = epip.tile([128, 256], F32, tag="E")
            Ev = E[:].rearrange("p (c h d) -> p c h d", c=4, h=4)
            nc.vector.tensor_mul(
                Ev[:, :, :, :],
                Tv[:, :, :, 0:16],
                Rv[:, :, :].unsqueeze(3).broadcast_to((128, 4, 4, 16)),
            )
            Ot = ps_bank1([CB, TT], "Ot")
            for c in range(4):
                mm(
                    Ot[:, 128 * c : 128 * c + 128],
                    E[:, 64 * c : 64 * c + 64],
                    ident[:],
                    is_transpose=True,
                    start=(c == 0), stop=(c == 3),
                )
            # un-window: attention + lepe into cat rows [64*br, 64*br+64)
            nc.vector.tensor_add(
                win_ap(cat, br, j, p0, CB),
                Ot[:].rearrange("c (h w) -> c h w", h=H),
                lepe[p0 : p0 + CB, :].rearrange("c (h w) -> c h w", h=H),
            )

    # ---------------- proj + residual ----------------
    for t in range(NT):
        sl = slice(TT * t, TT * t + TT)
        ap_ = ps_bank1([128, TT], "ap_")
        mm(ap_[:], _r(WpT[:]), _r(cat[:, sl]), start=True, stop=True)
        nc.vector.scalar_tensor_tensor(
            xf2[:, sl], ap_[:], bprojc[:], xT[:, sl], op0=OP.add, op1=OP.add
        )

    # ---------------- LN2 + MLP + residual ----------------
    for t in range(NT):
        sl = slice(TT * t, TT * t + TT)
        hn = layernorm_tile(xf2, t, "ln2")
        hp = ps_big4("hp")
        gel = gelp.tile([128, 4 * TT], FR, tag="gel")
        for hh in range(4):
            mm(
                hp[:, TT * hh : TT * hh + TT],
                _r(W1gT[:, 128 * hh : 128 * hh + 128]),
                _r(hn[:]),
                start=True, stop=True,
            )
            nc.scalar.activation(
                gel[:, TT * hh : TT * hh + TT],
                hp[:, TT * hh : TT * hh + TT],
                AF.Gelu,
                bias=gbcols[:, hh : hh + 1],
                scale=1.0,
            )
        o2 = ps_bank1([128, TT], "o2")
        for hh in range(4):
            mm(
                o2[:],
                _r(W2T[:, 128 * hh : 128 * hh + 128]),
                _r(gel[:, TT * hh : TT * hh + TT]),
                start=(hh == 0), stop=(hh == 3),
            )
        ot = outp.tile([128, TT], F32, tag="ot")
        nc.vector.scalar_tensor_tensor(
            ot[:], o2[:], b2c[:], xf2[:, sl], op0=OP.add, op1=OP.add
        )
        dma(io["out"][:, sl], ot[:])


_NC_CACHE = {}


def build_nc():
    key = "nc"
    if key in _NC_CACHE:
        return _NC_CACHE[key]
    nc = bacc.Bacc("TRN2", target_bir_lowering=False, debug=False)
    io = {}
    for name in INPUT_NAMES:
        io[name] = nc.dram_tensor(
            name, INPUT_SHAPES[name], F32, kind="ExternalInput"
        ).ap()
    io["out"] = nc.dram_tensor("out", [C, L], F32, kind="ExternalOutput").ap()
    with tile.TileContext(nc) as tc:
        with ExitStack() as ctx:
            emit(ctx, tc, io)
    nc.compile()
    _NC_CACHE[key] = nc
    return nc


def make_in_maps(inputs):
    in_maps = []
    for b in range(B):
        m = {
            "x": np.ascontiguousarray(
                inputs["x"][b].reshape(C, L).astype(np.float32)
            ),
            "z": np.ascontiguousarray(inputs["z"][b].astype(np.float32)),
        }
        for name in INPUT_NAMES:
            if name in ("x", "z"):
                continue
            m[name] = np.ascontiguousarray(np.asarray(inputs[name], np.float32))
        in_maps.append(m)
    return in_maps


def kernel(**inputs):
    nc = build_nc()
    in_maps = make_in_maps(inputs)
    res = bass_utils.run_bass_kernel_spmd(nc, in_maps, list(range(B)))
    out = np.stack([res.results[b]["out"].reshape(C, H, W) for b in range(B)])
    return out.astype(np.float32)


if __name__ == "__main__":
    # CoreSim numerics check of core 0 against the reference (dev only).
    import sys

    sys.path.insert(0, "/root/problem")
    import reference

    from concourse.bass_interp import CoreSim

    # CoreSim has no Gelu; patch it (HW has a native erf-gelu table).
    import scipy.special
    from concourse import bass_interp

    _orig_act = bass_interp.InstructionExecutor.visit_InstActivation

    def _patched_act(self, instruction, *, reg_snapshot=None):
        if instruction.func == mybir.ActivationFunctionType.Gelu:
            instruction.func = mybir.ActivationFunctionType.Identity
            try:
                _orig_act(self, instruction, reg_snapshot=reg_snapshot)
            finally:
                instruction.func = mybir.ActivationFunctionType.Gelu
            ov = self.view_ap(
                instruction.outs[0],
                bass_interp.Direction.WRITE,
                instruction,
                reg_snapshot=reg_snapshot,
            )
            x = ov.astype(np.float64)
            ov[:] = (
                x * 0.5 * (1.0 + scipy.special.erf(x / np.sqrt(2.0)))
            ).astype(np.float32)
            return
        return _orig_act(self, instruction, reg_snapshot=reg_snapshot)

    bass_interp.InstructionExecutor.visit_InstActivation = _patched_act

    inputs = {k: np.asarray(v) for k, v in reference.setup_inputs().items()}
    expected = np.asarray(reference.reference(**inputs))

    nc = build_nc()
    print("built+compiled", flush=True)
    sim = CoreSim(nc, require_finite=True, require_nnan=True)
    m = make_in_maps(inputs)[0]
    for k, v in m.items():
        sim.tensor(k)[:] = v
    sim.simulate(check_with_hw=False)
    got = sim.tensor("out").reshape(C, H, W)
    exp0 = expected[0]
    err = np.abs(got - exp0)
    denom = np.abs(exp0).max()
    print("absmax err:", err.max(), "rel:", err.max() / denom)
    print(
        "rms rel:",
        np.sqrt(((got - exp0) ** 2).mean()) / np.sqrt((exp0**2).mean()),
    )



# revision 15
# speedup vs baseline: 1.7040x; 1.7040x over previous
"""Trainium2 Bass kernel for DegradationRectifyNet block (CSWin-style window
attention + LePE depthwise conv + code-conditioned LN/MLP).

Data-parallel over batch B=8 across 8 NeuronCores; one image per core.
On-chip everything is channel-major (C on partitions, tokens on free dim).

v2 performance notes (vs the original baseline):
  - All big matmul moving operands are f32r or bf16 (the fp32 ones-matmul
    LN stats were paying a 4x cycles-per-row penalty).
  - LN variance is accumulated into a whole-image buffer and hit with ONE
    ACT Sqrt per layernorm: Sqrt/Exp/Gelu live in different ACT tables and
    each table switch costs 1.28us, so per-tile sqrt interleaved with
    exp/gelu thrashed tables. Square/Copy/Identity are in every table.
  - 1/C is folded into the ones stationary so the stats matmuls produce
    E[x], E[x^2] directly; 1/sd uses the fast approx reciprocal.
  - Attention internals (q/k/v, exp scores, AV stationary, epilogue
    transposes) are bf16: same PE rate, half the DVE/DMA bytes, and
    transposes drop from 2.0 to 1.0 cycles/row.
  - PSUM is retagged for pipelining: scores rotate through 2x[128,1024]
    banks with per-half exp, so the PE never waits on the ACT engine; the
    MLP hidden reuses the same tag with per-half gelu.
  - PE p-states: the tensor engine only reaches 2.4GHz after ~3us of
    continuous execution, so the whole schedule is organized to keep its
    queue non-empty (the baseline averaged ~1.1GHz).
"""

import numpy as np

import concourse.bass as bass
import concourse.bacc as bacc
import concourse.tile as tile
from concourse import mybir
from concourse import bass_utils
from concourse.masks import make_identity
from contextlib import ExitStack

F32 = mybir.dt.float32
F32R = mybir.dt.float32r
BF16 = mybir.dt.bfloat16
AF = mybir.ActivationFunctionType
OP = mybir.AluOpType

B = 8
C = 128          # channels
H = W = 64
L = H * W        # 4096 tokens
SS = 8           # split size
CB = C // 2      # branch channels = 64
NH = 4           # heads per branch
D = CB // NH     # head dim = 16
HID = 4 * C      # 512
CHZ = 512
NT = 8           # token tiles
TT = 512         # tokens per tile
NWIN = 8         # windows per branch
EPS = 1e-5

INPUT_NAMES = [
    "x", "z", "ln1_g", "ln1_b", "ln2_g", "ln2_b", "Wz1", "Wz2", "Wqkv",
    "Wproj", "bproj", "lepe_w0", "lepe_b0", "lepe_w1", "lepe_b1",
    "W1", "b1", "W2", "b2",
]
INPUT_SHAPES = {
    "x": [C, L], "z": [CHZ],
    "ln1_g": [C], "ln1_b": [C], "ln2_g": [C], "ln2_b": [C],
    "Wz1": [C, CHZ], "Wz2": [C, CHZ], "Wqkv": [3 * C, C],
    "Wproj": [C, C], "bproj": [C],
    "lepe_w0": [CB, 1, 3, 3], "lepe_b0": [CB],
    "lepe_w1": [CB, 1, 3, 3], "lepe_b1": [CB],
    "W1": [HID, C], "b1": [HID], "W2": [C, HID], "b2": [C],
}


def emit(ctx: ExitStack, tc: tile.TileContext, io: dict):
    nc = tc.nc

    consts = ctx.enter_context(tc.tile_pool(name="consts", bufs=1))
    wpool = ctx.enter_context(tc.tile_pool(name="wpool", bufs=1))
    big = ctx.enter_context(tc.tile_pool(name="big", bufs=1))
    h1p = ctx.enter_context(tc.tile_pool(name="h1p", bufs=2))
    msqp = ctx.enter_context(tc.tile_pool(name="msqp", bufs=2))
    padp = ctx.enter_context(tc.tile_pool(name="padp", bufs=2))
    expp = ctx.enter_context(tc.tile_pool(name="expp", bufs=2))
    v4p = ctx.enter_context(tc.tile_pool(name="v4p", bufs=2))
    epip = ctx.enter_context(tc.tile_pool(name="epip", bufs=2))
    kstp = ctx.enter_context(tc.tile_pool(name="kstp", bufs=1))
    gelp = ctx.enter_context(tc.tile_pool(name="gelp", bufs=2))
    outp = ctx.enter_context(tc.tile_pool(name="outp", bufs=2))

    # one PSUM pool, 4 tags totalling exactly 8 banks:
    #   sp   2 x [128,1024] f32 (4 banks): QK score halves / LN stats / MLP hidden
    #   avh  1 x [32,512]   f32 (1 bank):  per-head AV accumulation
    #   conv 1 x [128,512]  f32 (1 bank):  LePE conv accumulate
    #   lin  2 x [128,512]  f32 (2 banks): linears, weight/epilogue transposes
    psum = ctx.enter_context(tc.tile_pool(name="psum", bufs=1, space="PSUM"))

    def ps_sp(name):
        return psum.tile([128, 1024], F32, tag="sp", bufs=2, name=name)

    def ps_lin(shape, name, dtype=F32):
        return psum.tile(shape, dtype, tag="lin", bufs=2, name=name)

    dma = nc.sync.dma_start
    mm = nc.tensor.matmul

    # ---------------- input DMAs (x first: LN1 starts on it) --------------
    xT = big.tile([128, L], F32, tag="xT")
    for t in range(NT):
        dma(xT[:, TT * t : TT * t + TT], io["x"][:, TT * t : TT * t + TT])

    # weight staging slab: 20 [128,128] blocks, transposed later on PE
    wblocks = (
        [("Wqkv", io["Wqkv"][128 * j : 128 * j + 128, :]) for j in range(3)]
        + [("W1", io["W1"][128 * h : 128 * h + 128, :]) for h in range(4)]
        + [("W2", io["W2"][:, 128 * h : 128 * h + 128]) for h in range(4)]
        + [("Wproj", io["Wproj"][:, :])]
        + [("Wz1", io["Wz1"][:, 128 * k : 128 * k + 128]) for k in range(4)]
        + [("Wz2", io["Wz2"][:, 128 * k : 128 * k + 128]) for k in range(4)]
    )
    wst = wpool.tile([128, 128 * len(wblocks)], F32, tag="wst")
    for i, (_, src) in enumerate(wblocks):
        dma(wst[:, 128 * i : 128 * i + 128], src)

    def col(name):
        t = consts.tile([128, 1], F32, tag="col_" + name)
        dma(t[:], io[name].unsqueeze(1))
        return t

    g1c = col("ln1_g")
    bln1 = col("ln1_b")
    g2c = col("ln2_g")
    bln2 = col("ln2_b")
    bprojc = col("bproj")
    b2c = col("b2")

    b1cols = consts.tile([128, 4], F32, tag="b1cols")
    for h in range(4):
        dma(b1cols[:, h : h + 1], io["b1"][128 * h : 128 * h + 128].unsqueeze(1))
    zcols = consts.tile([128, 4], F32, tag="zcols")
    for k in range(4):
        dma(zcols[:, k : k + 1], io["z"][128 * k : 128 * k + 128].unsqueeze(1))

    # lepe bias as a column (rows 0:64 branch0, 64:128 branch1)
    lbias = consts.tile([128, 1], F32, tag="lbias")
    dma(lbias[0:CB, :], io["lepe_b0"].unsqueeze(1))
    dma(lbias[CB:128, :], io["lepe_b1"].unsqueeze(1))

    # conv tap weights: rows 0:64 branch-0 taps (a,b); rows 64:128 branch-1
    # taps transposed (branch-1 window images are stored transposed)
    wcomb = consts.tile([128, 9], F32, tag="wcomb")
    dma(wcomb[0:CB, :], io["lepe_w0"].rearrange("c o a b -> c (o a b)"))
    wtap = consts.tile([64, 9], F32, tag="wtap")
    dma(wtap[:], io["lepe_w1"].rearrange("c o a b -> c (o a b)"))

    # ---------------- constants ----------------
    ident = consts.tile([128, 128], F32, tag="ident")
    make_identity(nc, ident[:])
    identB = consts.tile([128, 128], BF16, tag="identB")
    nc.vector.tensor_copy(identB[:], ident[:])
    # 64x64 identity living at partitions 64:128 (for branch-1 v transposes)
    ident2f = consts.tile([128, 64], F32, tag="ident2f")
    nc.gpsimd.memset(ident2f[:], 0.0)
    nc.gpsimd.affine_select(
        out=ident2f[:], in_=ident2f[:], compare_op=OP.not_equal, fill=1.0,
        base=-64, pattern=[[-1, 64]], channel_multiplier=1,
    )
    ident2 = consts.tile([128, 64], BF16, tag="ident2")
    nc.vector.tensor_copy(ident2[:], ident2f[:])

    onesC = consts.tile([128, 128], F32, tag="onesC")
    nc.gpsimd.memset(onesC[:], 1.0 / C)
    # genuinely-rounded f32r copy: walrus requires f32r matmul operands to be
    # produced by an engine write with f32r output (bitcasts are rejected)
    onesCR = consts.tile([128, 128], F32R, tag="onesCR")
    nc.vector.tensor_copy(onesCR[:], onesC[:])
    epscol = consts.tile([128, 1], F32, tag="epscol")
    nc.gpsimd.memset(epscol[:], EPS)

    nc.gpsimd.tensor_copy(
        wcomb[CB:128, :].rearrange("c (b a) -> c b a", b=3),
        wtap[:].rearrange("c (a b) -> c a b", a=3).transpose([0, 2, 1]),
    )
    diags = []
    for k in range(9):
        dg = consts.tile([128, 128], BF16, tag=f"diag{k}")
        nc.vector.tensor_scalar_mul(dg[:], ident[:], wcomb[:, k : k + 1])
        diags.append(dg)

    # v4 template: 32-wide head slots, col 16 ones (denominator), rest zero
    v4f = consts.tile([128, TT], F32, tag="v4f")
    nc.gpsimd.memset(v4f[:], 0.0)
    nc.vector.memset(
        v4f[:].rearrange("p (s w) -> p s w", s=16)[:, :, 16:17], 1.0
    )
    v4br = []
    for brr in range(2):
        v4 = consts.tile([128, TT], BF16, tag=f"v4br{brr}")
        nc.vector.tensor_copy(v4[:], v4f[:])
        v4br.append(v4)

    # persistent QK stationary slabs: slab h keeps only head h's 16 rows live
    # per branch (rows 64*br+16*h), rest stay zero, so the 64-row contraction
    # reads start at partition base 0/64.
    kst4 = [
        kstp.tile([128, TT], BF16, tag=f"kst{i}", name=f"kst{i}")
        for i in range(NH)
    ]
    for i in range(NH):
        nc.gpsimd.memset(kst4[i][:], 0.0)

    # ---------------- big activation buffers ----------------
    qT = big.tile([128, L], BF16, tag="qT")
    kT = big.tile([128, L], BF16, tag="kT")
    vT = big.tile([128, L], BF16, tag="vT")
    cat = big.tile([128, L], BF16, tag="cat")
    xf2 = big.tile([128, L], F32R, tag="xf2")
    xc4 = big.tile([128, L], F32, tag="xc4")     # x - mean
    var4 = big.tile([128, L], F32, tag="var4")   # variance, then 1/sd
    sd4 = big.tile([128, L], F32, tag="sd4")     # sd scratch

    # ---------------- layernorm: per-tile stats into shared buffers -------
    def ln_stats_tile(src, t):
        # src dtype decides the E[x] matmul rate: f32r xf2 runs at 1 cycle/row,
        # the DMA'd f32 xT pays 4x on its sum (only 8 such matmuls).
        sl = slice(TT * t, TT * t + TT)
        xt = src[:, sl]
        xsq = msqp.tile([128, TT], F32R, tag="xsq")
        nc.gpsimd.tensor_mul(xsq[:], xt, xt)
        s12 = ps_sp("s12")
        mps = s12[:, 0:TT]
        s2ps = s12[:, TT : 2 * TT]
        ones1 = onesC[:] if xt.dtype == F32 else onesCR[:]
        mm(mps, ones1, xt, start=True, stop=True)
        mm(s2ps, onesCR[:], xsq[:], start=True, stop=True)
        msq = msqp.tile([128, TT], F32, tag="msq")
        nc.scalar.activation(msq[:], mps, AF.Square)
        nc.vector.tensor_sub(var4[:, sl], s2ps, msq[:])
        nc.vector.tensor_sub(xc4[:, sl], xt, mps)

    def ln_finish():
        nc.scalar.activation(sd4[:], var4[:], AF.Sqrt, bias=epscol[:])
        nc.vector.reciprocal_approx_fast(var4[:], sd4[:])  # var4 becomes 1/sd

    # ---------------- phase 1: LN1 stats (overlaps x DMA) -----------------
    for t in range(NT):
        ln_stats_tile(xT, t)

    # ---------------- weight transposes (PE busy during LN tail) ---------
    WgT = wpool.tile([128, 3 * C], F32R, tag="WgT")
    WqkvT = wpool.tile([128, 3 * C], F32, tag="WqkvT")
    W1gT = wpool.tile([128, HID], F32R, tag="W1gT")
    W1T = wpool.tile([128, HID], F32, tag="W1T")
    W2T = wpool.tile([128, HID], BF16, tag="W2T")
    WpT = wpool.tile([128, C], BF16, tag="WpT")
    Wz1T = wpool.tile([128, CHZ], F32, tag="Wz1T")
    Wz2T = wpool.tile([128, CHZ], F32, tag="Wz2T")

    wdsts = (
        [(WgT[:, 128 * j : 128 * j + 128], g1c,
          WqkvT[:, 128 * j : 128 * j + 128]) for j in range(3)]
        + [(W1gT[:, 128 * h : 128 * h + 128], g2c,
            W1T[:, 128 * h : 128 * h + 128]) for h in range(4)]
        + [(W2T[:, 128 * h : 128 * h + 128], None, None) for h in range(4)]
        + [(WpT[:], None, None)]
        + [(Wz1T[:, 128 * k : 128 * k + 128], None, None) for k in range(4)]
        + [(Wz2T[:, 128 * k : 128 * k + 128], None, None) for k in range(4)]
    )
    for i, (dst, scale_col, copy_dst) in enumerate(wdsts):
        pt = ps_lin([128, 512], "ptw")[:, 0:128]
        nc.tensor.transpose(pt, wst[:, 128 * i : 128 * i + 128], ident[:])
        if scale_col is not None:
            nc.vector.tensor_scalar_mul(dst, pt, scale_col[:])
        else:
            nc.vector.tensor_copy(dst, pt)
        if copy_dst is not None:
            nc.vector.tensor_copy(copy_dst, pt)

    # ---------------- z-conditioned bias columns ----------------
    def zbias(WzT, blnc, tag):
        zp = ps_lin([128, 512], "zp")[:, 0:1]
        for k in range(4):
            mm(
                zp, WzT[:, 128 * k : 128 * k + 128], zcols[:, k : k + 1],
                start=(k == 0), stop=(k == 3),
            )
        bz = consts.tile([128, 1], F32, tag=tag)
        nc.vector.tensor_add(bz[:], zp, blnc[:])
        return bz

    bz1 = zbias(Wz1T, bln1, "bz1")
    bz2 = zbias(Wz2T, bln2, "bz2")

    w0cols = consts.tile([128, 3], F32, tag="w0cols")
    for j in range(3):
        wp = ps_lin([128, 512], "wp")[:, 0:1]
        mm(wp, WqkvT[:, 128 * j : 128 * j + 128], bz1[:], start=True, stop=True)
        nc.vector.tensor_copy(w0cols[:, j : j + 1], wp)
    gbcols = consts.tile([128, 4], F32, tag="gbcols")
    for h in range(4):
        wp = ps_lin([128, 512], "wp")[:, 0:1]
        mm(wp, W1T[:, 128 * h : 128 * h + 128], bz2[:], start=True, stop=True)
        nc.vector.tensor_add(gbcols[:, h : h + 1], wp, b1cols[:, h : h + 1])

    ln_finish()  # LN1 sqrt + approx recip (ACT/DVE; PE runs transposes above)

    # ---------------- phase 2: qkv projections ----------------
    for t in range(NT):
        sl = slice(TT * t, TT * t + TT)
        hn = h1p.tile([128, TT], F32R, tag="hn")
        nc.vector.tensor_mul(hn[:], xc4[:, sl], var4[:, sl])
        for j, dstT in enumerate((qT, kT, vT)):
            qp = ps_lin([128, 512], "qp")
            mm(qp[:], WgT[:, 128 * j : 128 * j + 128], hn[:],
               start=True, stop=True)
            nc.scalar.activation(
                dstT[:, sl], qp[:], AF.Identity, bias=w0cols[:, j : j + 1]
            )

    # window access patterns -------------------------------------------------
    # branch 0: vertical strip window j = cols [8j,8j+8); raster (h, w)
    # branch 1: horizontal strip window j, stored transposed; raster (w, h)
    def win_ap(src, br, j, p0, psz):
        a3 = src[p0 : p0 + psz, :].rearrange("c (h w) -> c h w", h=H)
        if br == 0:
            return a3[:, :, SS * j : SS * j + SS]
        return a3[:, SS * j : SS * j + SS, :].transpose([0, 2, 1])

    # ---------------- phase 3: attention + lepe, one window pair per j ----
    for j in range(NWIN):
        # LePE: zero-padded q window images (66 x 10), both branches stacked
        pad = padp.tile([128, 660], BF16, tag="pad")
        nc.gpsimd.memset(pad[:], 0.0)
        pad3 = pad[:].rearrange("c (h w) -> c h w", h=66)
        nc.gpsimd.tensor_copy(pad3[0:CB, 1:65, 1:9], win_ap(qT, 0, j, 0, CB))
        nc.gpsimd.tensor_copy(pad3[CB:128, 1:65, 1:9], win_ap(qT, 1, j, CB, CB))
        vst = v4p.tile([128, TT], BF16, tag="vst")
        dma(
            vst[0:CB, :].rearrange("c (a b) -> c a b", a=64),
            win_ap(vT, 0, j, 0, CB),
        )
        dma(vst[CB:128, :], vT[CB:128, TT * j : TT * j + TT])
        convp = psum.tile([128, TT], F32, tag="conv", bufs=1, name="convp")
        taps = [(a, b) for a in (-1, 0, 1) for b in (-1, 0, 1)]
        for idx, (a, b) in enumerate(taps):
            src = pad3[:, 1 + a : 65 + a, 1 + b : 9 + b]
            mm(
                convp[:], diags[3 * (a + 1) + (b + 1)][:], src,
                start=(idx == 0), stop=(idx == 8),
            )
        lepe = padp.tile([128, TT], F32, tag="lepe")
        nc.vector.tensor_copy(lepe[:], convp[:])

        for br in range(2):
            p0 = CB * br
            # v': token-major v (via PE transpose), into 32-wide head slots
            vps = ps_lin([128, 256], "vps", dtype=BF16)
            idv = identB[0:CB, 0:CB] if br == 0 else ident2[CB:128, :]
            for c in range(4):
                mm(
                    vps[:, 64 * c : 64 * c + 64],
                    vst[p0 : p0 + CB, 128 * c : 128 * c + 128],
                    idv,
                    is_transpose=True,
                    start=(c == 0), stop=(c == 3),
                )
            v4 = v4br[br]
            nc.vector.tensor_copy(
                v4[:].rearrange("p (c h s) -> p c h s", c=4, h=4)[:, :, :, 0:16],
                vps[:].rearrange("p (c h d) -> p c h d", c=4, h=4),
            )

            # fp32r matmuls cannot write PSUM at a partition offset, so each
            # head's AV accumulates at offset 0 and is copied to its A stripe.
            A = epip.tile([128, TT], BF16, tag="A")
            for h in range(NH):
                hp0 = p0 + D * h
                kst = kst4[h]
                if br == 0:
                    dma(
                        kst[hp0 : hp0 + D, :].rearrange("c (a b) -> c a b", a=64),
                        win_ap(kT, 0, j, hp0, D),
                    )
                else:
                    dma(
                        kst[hp0 : hp0 + D, :],
                        kT[hp0 : hp0 + D, TT * j : TT * j + TT],
                    )
                es = expp.tile([128, 4 * TT], BF16, tag="es")
                for half in range(2):
                    sph = ps_sp("sph")
                    for cc in range(2):
                        c = 2 * half + cc
                        mm(
                            sph[:, TT * cc : TT * cc + TT],
                            kst[p0 : p0 + CB, 128 * c : 128 * c + 128],
                            win_ap(qT, br, j, p0, CB),
                            start=True, stop=True,
                        )
                    nc.scalar.activation(
                        es[:, 1024 * half : 1024 * half + 1024],
                        sph[:],
                        AF.Exp,
                        scale=float(D) ** -0.5,
                    )
                avh = psum.tile([32, TT], F32, tag="avh", bufs=1, name="avh")
                for c in range(4):
                    mm(
                        avh[:],
                        v4[:, 128 * c + 32 * h : 128 * c + 32 * h + 32],
                        es[:, TT * c : TT * c + TT],
                        start=(c == 0), stop=(c == 3),
                    )
                nc.vector.tensor_copy(A[32 * h : 32 * h + 32, :], avh[:])

            # epilogue: transpose -> divide by denominators -> transpose back
            Tb = ps_lin([128, TT], "Tb", dtype=BF16)
            for c in range(4):
                mm(
                    Tb[:, 128 * c : 128 * c + 128],
                    A[:, 128 * c : 128 * c + 128],
                    identB[:],
                    is_transpose=True,
                    start=(c == 0), stop=(c == 3),
                )
            Tv = Tb.rearrange("p (c h s) -> p c h s", c=4, h=4)
            R = epip.tile([128, 16], F32, tag="R")
            Rv = R[:].rearrange("p (c h) -> p c h", c=4)
            nc.vector.reciprocal(Rv[:, :, :], Tv[:, :, :, 16])
            E = epip.tile([128, 256], BF16, tag="E")
            Ev = E[:].rearrange("p (c h d) -> p c h d", c=4, h=4)
            nc.vector.tensor_mul(
                Ev[:, :, :, :],
                Tv[:, :, :, 0:16],
                Rv[:, :, :].unsqueeze(3).broadcast_to((128, 4, 4, 16)),
            )
            Ot = ps_lin([CB, TT], "Ot", dtype=BF16)
            for c in range(4):
                mm(
                    Ot[:, 128 * c : 128 * c + 128],
                    E[:, 64 * c : 64 * c + 64],
                    identB[:],
                    is_transpose=True,
                    start=(c == 0), stop=(c == 3),
                )
            # un-window: (attention + lepe_bias) + lepe into cat rows
            nc.vector.scalar_tensor_tensor(
                win_ap(cat, br, j, p0, CB),
                Ot[:].rearrange("c (h w) -> c h w", h=H),
                lbias[p0 : p0 + CB, :],
                lepe[p0 : p0 + CB, :].rearrange("c (h w) -> c h w", h=H),
                op0=OP.add, op1=OP.add,
            )

    # ---------------- phase 4: proj + residual + LN2 stats ----------------
    for t in range(NT):
        sl = slice(TT * t, TT * t + TT)
        ap_ = ps_lin([128, 512], "ap_")
        mm(ap_[:], WpT[:], cat[:, sl], start=True, stop=True)
        nc.vector.scalar_tensor_tensor(
            xf2[:, sl], ap_[:], bprojc[:], xT[:, sl], op0=OP.add, op1=OP.add
        )
        ln_stats_tile(xf2, t)
    ln_finish()

    # ---------------- phase 5: MLP + residual ----------------
    for t in range(NT):
        sl = slice(TT * t, TT * t + TT)
        hn = h1p.tile([128, TT], F32R, tag="hn")
        nc.vector.tensor_mul(hn[:], xc4[:, sl], var4[:, sl])
        gel = gelp.tile([128, 4 * TT], BF16, tag="gel")
        for half in range(2):
            hp = ps_sp("hp")
            for hh2 in range(2):
                hh = 2 * half + hh2
                mm(
                    hp[:, TT * hh2 : TT * hh2 + TT],
                    W1gT[:, 128 * hh : 128 * hh + 128],
                    hn[:],
                    start=True, stop=True,
                )
                nc.scalar.activation(
                    gel[:, TT * hh : TT * hh + TT],
                    hp[:, TT * hh2 : TT * hh2 + TT],
                    AF.Gelu,
                    bias=gbcols[:, hh : hh + 1],
                )
        o2 = ps_lin([128, 512], "o2")
        for hh in range(4):
            mm(
                o2[:],
                W2T[:, 128 * hh : 128 * hh + 128],
                gel[:, TT * hh : TT * hh + TT],
                start=(hh == 0), stop=(hh == 3),
            )
        ot = outp.tile([128, TT], F32, tag="ot")
        nc.vector.scalar_tensor_tensor(
            ot[:], o2[:], b2c[:], xf2[:, sl], op0=OP.add, op1=OP.add
        )
        dma(io["out"][:, sl], ot[:])


_NC_CACHE = {}


def build_nc():
    key = "nc"
    if key in _NC_CACHE:
        return _NC_CACHE[key]
    nc = bacc.Bacc("TRN2", target_bir_lowering=False, debug=False)
    io = {}
    for name in INPUT_NAMES:
        io[name] = nc.dram_tensor(
            name, INPUT_SHAPES[name], F32, kind="ExternalInput"
        ).ap()
    io["out"] = nc.dram_tensor("out", [C, L], F32, kind="ExternalOutput").ap()
    with tile.TileContext(nc) as tc:
        with ExitStack() as ctx:
            emit(ctx, tc, io)
    nc.compile()
    _NC_CACHE[key] = nc
    return nc


def make_in_maps(inputs):
    in_maps = []
    for b in range(B):
        m = {
            "x": np.ascontiguousarray(
                inputs["x"][b].reshape(C, L).astype(np.float32)
            ),
            "z": np.ascontiguousarray(inputs["z"][b].astype(np.float32)),
        }
        for name in INPUT_NAMES:
            if name in ("x", "z"):
                continue
            m[name] = np.ascontiguousarray(np.asarray(inputs[name], np.float32))
        in_maps.append(m)
    return in_maps


def kernel(**inputs):
    nc = build_nc()
    in_maps = make_in_maps(inputs)
    res = bass_utils.run_bass_kernel_spmd(nc, in_maps, list(range(B)))
    out = np.stack([res.results[b]["out"].reshape(C, H, W) for b in range(B)])
    return out.astype(np.float32)


if __name__ == "__main__":
    # CoreSim numerics check of core 0 against the reference (dev only).
    import sys

    sys.path.insert(0, "/root/problem")
    import reference

    from concourse.bass_interp import CoreSim

    # CoreSim has no Gelu; patch it (HW has a native erf-gelu table).
    import scipy.special
    from concourse import bass_interp

    _orig_act = bass_interp.InstructionExecutor.visit_InstActivation

    def _patched_act(self, instruction, *, reg_snapshot=None):
        if instruction.func == mybir.ActivationFunctionType.Gelu:
            instruction.func = mybir.ActivationFunctionType.Identity
            try:
                _orig_act(self, instruction, reg_snapshot=reg_snapshot)
            finally:
                instruction.func = mybir.ActivationFunctionType.Gelu
            ov = self.view_ap(
                instruction.outs[0],
                bass_interp.Direction.WRITE,
                instruction,
                reg_snapshot=reg_snapshot,
            )
            x = ov.astype(np.float64)
            ov[:] = (
                x * 0.5 * (1.0 + scipy.special.erf(x / np.sqrt(2.0)))
            ).astype(np.float32)
            return
        return _orig_act(self, instruction, reg_snapshot=reg_snapshot)

    bass_interp.InstructionExecutor.visit_InstActivation = _patched_act

    inputs = {k: np.asarray(v) for k, v in reference.setup_inputs().items()}
    expected = np.asarray(reference.reference(**inputs))

    nc = build_nc()
    print("built+compiled", flush=True)
    sim = CoreSim(nc, require_finite=True, require_nnan=True)
    m = make_in_maps(inputs)[0]
    for k, v in m.items():
        sim.tensor(k)[:] = v
    sim.simulate(check_with_hw=False)
    got = sim.tensor("out").reshape(C, H, W)
    exp0 = expected[0]
    err = np.abs(got - exp0)
    denom = np.abs(exp0).max()
    print("absmax err:", err.max(), "rel:", err.max() / denom)
    print(
        "rms rel:",
        np.sqrt(((got - exp0) ** 2).mean()) / np.sqrt((exp0**2).mean()),
    )
